# revision 27
# baseline (speedup 1.0000x reference)
"""Multi-Head Latent Attention kernel for 8 Trainium2 NeuronCores.

Sharding: 8 cores = 2 (batch) x 4 (head groups of 4 heads).
Each core computes, for its (batch b, head group g):
  - kv = x_b @ Wc + bc              (replicated small compressor)
  - k,v,q projections for its 4 heads (column-parallel)
  - causal attention for its 4 heads (transpose-free: S^T layout)
  - partial out = y_heads @ Wo[rows of g]   (row-parallel)
Host sums the 4 partials per batch and adds bo.

All matmuls run in bf16 with fp32 PSUM accumulation. Softmax runs
without max-subtraction (scores for this problem are O(1)). The
denominator is REPLICATED across 64 PSUM partitions for free by
augmenting V with a 64-wide block of ones columns (M=128 instead of
65; matmul time depends only on streamed columns), so the normalize
is reciprocal + muls with no cross-partition broadcast. k-bias is
dropped entirely (softmax is invariant to per-query constants:
q.bk is constant across keys).

Scheduling: all projection/out-proj work is split into ~single-matmul
micro-closures drip-fed between the scores and PV matmuls of the
attention ki-loop, keeping PE dense (p-state ramps to full clock).
The normalize is column-split: the first half of each query window is
final a few key-tiles before the window ends, so only a short
second-half chain sits on the window boundary.
"""
import sys
import math

sys.path.insert(0, "/opt/trn_rl_repo")

import numpy as np
import ml_dtypes

import concourse.bass as bass
import concourse.tile as tile
from concourse import bacc, mybir
from concourse.bass_utils import run_bass_kernel_spmd

BF16 = ml_dtypes.bfloat16

# Problem shape (hardcoded per contract)
B, T, D = 2, 2048, 1024
H = 16
HD = 64           # head dim
KV = 16           # latent dim
HPC = 4           # heads per core
GD = HPC * HD     # head-group width = 256
NKT = T // 128    # key tiles = 16
SCALE = 1.0 / math.sqrt(HD)

F32 = mybir.dt.float32
BF = mybir.dt.bfloat16

_CACHE = {}
DEBUG = False


def _build_program():
    nc = bacc.Bacc("TRN2", target_bir_lowering=False, debug=False)

    xT = nc.dram_tensor("xT", [D, T], BF, kind="ExternalInput")
    wq = nc.dram_tensor("wq", [D, GD], BF, kind="ExternalInput")
    bq = nc.dram_tensor("bq", [128, 2], F32, kind="ExternalInput")
    wc = nc.dram_tensor("wc", [128, 8 * KV], BF, kind="ExternalInput")
    bc = nc.dram_tensor("bc", [KV, 1], F32, kind="ExternalInput")
    wk = nc.dram_tensor("wk", [KV, GD], BF, kind="ExternalInput")
    wv = nc.dram_tensor("wv", [KV, GD], BF, kind="ExternalInput")
    bv = nc.dram_tensor("bv", [1, GD], BF, kind="ExternalInput")
    wo = nc.dram_tensor("wo", [GD, D], BF, kind="ExternalInput")
    tri = nc.dram_tensor("tri", [128, 128], BF, kind="ExternalInput")
    outp = nc.dram_tensor("outp", [T, D], BF, kind="ExternalOutput")
    if DEBUG:
        dbg_kv = nc.dram_tensor("dbg_kv", [KV, T], BF, kind="ExternalOutput")
        dbg_k = nc.dram_tensor("dbg_k", [128, 2 * T], BF, kind="ExternalOutput")
        dbg_q = nc.dram_tensor("dbg_q", [128, 2 * T], BF, kind="ExternalOutput")
        dbg_yn = nc.dram_tensor("dbg_yn", [128, 2 * T], BF, kind="ExternalOutput")
        dbg_v = nc.dram_tensor("dbg_v", [128, NKT * HPC * 128], BF,
                               kind="ExternalOutput")

    EXP = mybir.ActivationFunctionType.Exp

    with tile.TileContext(nc) as tc:
        with (
            tc.tile_pool(name="const", bufs=1) as const,
            tc.tile_pool(name="work", bufs=3) as work,
            tc.tile_pool(name="pexps", bufs=16) as pexps,
            tc.tile_pool(name="ostg", bufs=4) as ostg,
            tc.tile_pool(name="ps", bufs=2, space="PSUM") as ps,
        ):
            # ---- DMA issue plan. kv(0) needs only wc + xts[0]; its drain
            # needs bc; k needs wk; q needs wq+bq; v needs wv+bv; diag
            # needs tri. Priority-ordered so the kv chain starts ~3us in.
            # scalar only issues tri+bv (so exp can start ~1.5us in);
            # gpsimd only slab0-odd + wq (so tri-muls start early).
            wc_sb = const.tile([128, 8, KV], BF)
            nc.sync.dma_start(out=wc_sb, in_=wc.ap().rearrange("p (k m) -> p k m", m=KV))

            xts = []
            xT_r = xT.ap().rearrange("(k p) t -> p k t", p=128)
            for n in range(4):
                xts.append(const.tile([128, 8, 512], BF, name=f"xts{n}"))
            for kt in range(8):
                eng = nc.sync if kt % 2 == 0 else nc.gpsimd
                eng.dma_start(out=xts[0][:, kt, :], in_=xT_r[:, kt, 0:512])
            wq_sb = const.tile([128, 8, GD], BF)
            nc.gpsimd.dma_start(
                out=wq_sb, in_=wq.ap().rearrange("(k p) m -> p k m", p=128))

            bc_sb = const.tile([KV, 1], F32)
            nc.sync.dma_start(out=bc_sb, in_=bc.ap())
            wk_sb = const.tile([KV, GD], BF)
            nc.sync.dma_start(out=wk_sb, in_=wk.ap())
            wv_sb = const.tile([KV, GD], BF)
            nc.sync.dma_start(out=wv_sb, in_=wv.ap())
            bq_sb = const.tile([128, 2, 1], F32)
            nc.sync.dma_start(out=bq_sb, in_=bq.ap().rearrange("p (c o) -> p c o", o=1))

            tri_sb = const.tile([128, 128], BF)
            nc.scalar.dma_start(out=tri_sb, in_=tri.ap())
            bvbc_sb = const.tile([128, GD], BF)
            bv_row = bv.ap()
            bv_bcast = bass.AP(tensor=bv_row.tensor, offset=bv_row.offset,
                               ap=[[0, 128]] + list(bv_row.ap)[1:])
            nc.scalar.dma_start(out=bvbc_sb, in_=bv_bcast)

            for kt in range(8):
                eng = nc.sync if kt % 2 == 0 else nc.gpsimd
                eng.dma_start(out=xts[1][:, kt, :], in_=xT_r[:, kt, 512:1024])
            for kt in range(0, 8, 2):
                nc.sync.dma_start(
                    out=xts[2][:, kt:kt + 2, :],
                    in_=xT_r[:, kt:kt + 2, 1024:1536])
            for kt in range(0, 8, 2):
                nc.sync.dma_start(
                    out=xts[3][:, kt:kt + 2, :],
                    in_=xT_r[:, kt:kt + 2, 1536:2048])
            wo_sb = const.tile([128, 2, D], BF)
            wo_r = wo.ap().rearrange("(k p) n -> p k n", p=128)
            for kt in range(2):
                nc.sync.dma_start(out=wo_sb[:, kt, :], in_=wo_r[:, kt, :])

            kvT_sb = const.tile([KV, T], BF)
            kT_sb = const.tile([128, 2, T], BF)
            qT_sb = const.tile([128, 2, T], BF)
            ynT_sb = const.tile([128, 2, T], BF)
            # v_sb per (key-tile, head): [v(64) | ones(64)] so the PV
            # matmul (M=128) lands y on partitions 0:64 and the softmax
            # denominator REPLICATED on partitions 64:128.
            v_sb = const.tile([128, NKT, HPC, 128], BF)
            nc.vector.memset(v_sb[:, 0:4, :, HD:128], 1.0)
            nc.gpsimd.memset(v_sb[:, 4:16, :, HD:128], 1.0)

            # ---- micro-unit emitters: each returns a list of closures,
            # ~one 512-col matmul each. PSUM tiles allocate at fire time.
            def emit_kv(ns_):
                """kv compress for one or two 512-token slabs. With two,
                the slabs pack at PE col positions 0/32 (M=16 each) and
                their streams run concurrently (col tiling)."""
                cell = {}

                def mm(kt):
                    if kt == 0:
                        cell["p"] = ps.tile([128, 512], F32, tag="bank",
                                            bufs=2, name=f"pkv{ns_[0]}")
                    for j, n in enumerate(ns_):
                        nc.tensor.matmul(
                            cell["p"][32 * j:32 * j + KV, :],
                            lhsT=wc_sb[:, kt, :], rhs=xts[n][:, kt, :],
                            start=(kt == 0), stop=(kt == 7),
                            tile_position=(0, 32 * j),
                        )
                    if kt == 7:
                        for j, n in enumerate(ns_):
                            nc.vector.tensor_scalar_add(
                                kvT_sb[:, n * 512:(n + 1) * 512],
                                cell["p"][32 * j:32 * j + KV, :], bc_sb)
                return [lambda kt=kt: (mm(kt), mm(kt + 1))
                        for kt in range(0, 8, 2)]

            def emit_k(c, n):
                def mm():
                    ns = slice(n * 512, n * 512 + 512)
                    pk = ps.tile([128, 512], F32, tag="bank", bufs=2,
                                 name=f"pk{c}{n}")
                    nc.tensor.matmul(
                        pk, lhsT=wk_sb[:, c * 128:(c + 1) * 128],
                        rhs=kvT_sb[:, ns], start=True, stop=True,
                    )
                    nc.vector.tensor_copy(out=kT_sb[:, c, ns], in_=pk)
                return [mm]

            def emit_q(c, n):
                cell = {}
                ns = slice(n * 512, n * 512 + 512)

                def mm(kt):
                    if kt == 0:
                        cell["p"] = ps.tile([128, 512], F32, tag="bank",
                                            bufs=2, name=f"pq{c}{n}")
                    nc.tensor.matmul(
                        cell["p"], lhsT=wq_sb[:, kt, c * 128:(c + 1) * 128],
                        rhs=xts[n][:, kt, :], start=(kt == 0), stop=(kt == 7),
                    )
                    if kt == 7:
                        nc.vector.tensor_scalar_add(
                            qT_sb[:, c, ns], cell["p"], bq_sb[:, c, :])
                return [lambda kt=kt: (mm(kt), mm(kt + 1))
                        for kt in range(0, 8, 2)]

            def emit_v(t):
                def mm():
                    pv = ps.tile([128, 512], F32, tag="bank", bufs=2,
                                 name=f"pv{t}")
                    nc.tensor.matmul(
                        pv[:, 0:GD], lhsT=kvT_sb[:, t * 128:(t + 1) * 128],
                        rhs=wv_sb, start=True, stop=True,
                    )
                    nc.vector.tensor_add(
                        out=v_sb[:, t, :, 0:HD],
                        in0=pv[:, 0:GD].rearrange("p (h d) -> p h d", h=HPC),
                        in1=bvbc_sb.rearrange("p (h d) -> p h d", h=HPC),
                    )
                return [mm]

            MID_ENGS = [nc.sync, nc.gpsimd]
            TAIL_ENGS = [nc.sync, nc.gpsimd, nc.scalar]

            def emit_outproj(qs, n, st_eng=0, tail=False, drain_sc=False):
                """out rows [qs, qs+128), cols [n*512, (n+1)*512): two
                micro-units (one matmul each; drain+store on the second)."""
                cell = {}

                def mm0():
                    cell["p"] = ps.tile([128, 512], F32, tag="bank", bufs=2,
                                        name=f"po{qs}_{n}")
                    nc.tensor.matmul(
                        cell["p"], lhsT=ynT_sb[:, 0, qs:qs + 128],
                        rhs=wo_sb[:, 0, n * 512:(n + 1) * 512],
                        start=True, stop=False,
                    )

                def mm1():
                    po = cell["p"]
                    nc.tensor.matmul(
                        po, lhsT=ynT_sb[:, 1, qs:qs + 128],
                        rhs=wo_sb[:, 1, n * 512:(n + 1) * 512],
                        start=False, stop=True,
                    )
                    st = ostg.tile([128, 512], BF, tag="ostg",
                                   name=f"ost{qs}_{n}")
                    if drain_sc:
                        nc.scalar.copy(st, po)
                    else:
                        nc.vector.tensor_copy(st, po)
                    engs = TAIL_ENGS if tail else MID_ENGS
                    engs[st_eng % len(engs)].dma_start(
                        out=outp.ap()[qs:qs + 128, n * 512:(n + 1) * 512],
                        in_=st,
                    )
                return [mm0, mm1]

            tri_ctr = [0]

            def emit_attn(q0, qw, pair, fillers=None, late_fillers=None,
                          tail_fillers=None):
                """Causal attention for queries [q0, q0+qw), heads 2*pair,
                2*pair+1. fillers: micro-closures paced across the ki loop
                between scores and PV. late_fillers fire right after the
                first-half normalize."""
                fillers = list(fillers or [])
                late = list(late_fillers or [])
                n_ki = (q0 + qw) // 128
                half = qw // 2
                ki_half = n_ki - (qw // 256) - 1
                tag = f"{q0}_{pair}"
                y_ps = ps.tile([128, 2, 512], F32, tag="yb", bufs=1,
                               name=f"yps{tag}")

                def normalize(c0, c1):
                    dn = work.tile([64, 2, 512], F32, tag="dn",
                                   name=f"dn{tag}_{c0}")
                    nc.vector.tensor_copy(
                        out=dn[:, :, c0:c1], in_=y_ps[64:128, :, c0:c1])
                    rbc = work.tile([64, 2, 512], F32, tag="rbc",
                                    name=f"rbc{tag}_{c0}")
                    nc.vector.reciprocal_approx_fast(
                        out=rbc[:, :, c0:c1], in_=dn[:, :, c0:c1])
                    for h in range(2):
                        nc.vector.tensor_mul(
                            ynT_sb[h * 64:(h + 1) * 64, pair,
                                   q0 + c0:q0 + c1],
                            y_ps[0:64, h, c0:c1], rbc[:, h, c0:c1],
                        )

                total = len(fillers)
                popped = 0
                prev = None
                for ki in range(n_ki):
                    vs = max(0, 128 * ki - q0)
                    s_ps = ps.tile([128, 2, 512], F32, tag="sc", bufs=2,
                                   name=f"s{tag}_{ki}")
                    for h in range(2):
                        base = h * 64
                        nc.tensor.matmul(
                            s_ps[:, h, vs:qw],
                            lhsT=kT_sb[base:base + 64, pair,
                                       ki * 128:(ki + 1) * 128],
                            rhs=qT_sb[base:base + 64, pair, q0 + vs:q0 + qw],
                            start=True, stop=True,
                            tile_position=(base, 0),
                        )
                    px = pexps.tile([128, 2, 512], BF, tag="pexp",
                                    name=f"px{tag}_{ki}")
                    nc.scalar.activation(px[:, :, vs:qw], s_ps[:, :, vs:qw], EXP)
                    if ki * 128 >= q0:
                        tri_b = bass.AP(tensor=tri_sb.tensor, offset=tri_sb.offset,
                                        ap=[list(tri_sb.ap)[0], [0, 2], [1, 128]])
                        eng = nc.gpsimd if tri_ctr[0] % 2 == 0 else nc.vector
                        tri_ctr[0] += 1
                        eng.tensor_mul(
                            px[:, :, vs:vs + 128],
                            px[:, :, vs:vs + 128], tri_b,
                        )
                    # paced micro-fillers between scores and the lagged PV
                    # (capped per slot so a burst never delays the PV)
                    tgt = min((total * (ki + 1) + n_ki - 1) // n_ki,
                              popped + 3)
                    while popped < tgt:
                        fillers.pop(0)()
                        popped += 1
                    if prev is not None:
                        pki, ppx, pvs = prev
                        for h in range(2):
                            nc.tensor.matmul(
                                y_ps[:, h, pvs:qw],
                                lhsT=v_sb[:, pki, 2 * pair + h, :],
                                rhs=ppx[:, h, pvs:qw],
                                start=(pki == 0), stop=False,
                            )
                        if pki == ki_half:
                            normalize(0, half)
                            # late fillers join the paced queue (they
                            # depend on the first-half normalize)
                            fillers.extend(late)
                            total += len(late)
                            late = []
                    prev = (ki, px, vs)
                pki, ppx, pvs = prev
                for h in range(2):
                    nc.tensor.matmul(
                        y_ps[:, h, pvs:qw],
                        lhsT=v_sb[:, pki, 2 * pair + h, :],
                        rhs=ppx[:, h, pvs:qw],
                        start=(pki == 0), stop=True,
                    )
                normalize(half, qw)
                for f in fillers:       # leftovers past the pop cap
                    f()
                for f in (tail_fillers or []):
                    f()

            # ---- emission schedule ----
            def proj_group(n, kv_ns=None):
                units = emit_kv(kv_ns) if kv_ns else []
                units += emit_k(0, n) + emit_k(1, n)
                units += emit_q(0, n) + emit_q(1, n)
                for t in range(4 * n, 4 * n + 4):
                    units += emit_v(t)
                return units

            def outproj_group(qc):
                units = []
                for i, (m, n) in enumerate((m, n) for m in range(4)
                                           for n in range(2)):
                    units += emit_outproj(qc * 512 + m * 128, n, st_eng=i)
                return units

            for u in proj_group(0, kv_ns=[0]):
                u()
            g1 = proj_group(1, kv_ns=[1])
            emit_attn(0, 512, 0, fillers=g1[:9])
            emit_attn(0, 512, 1, fillers=g1[9:])
            emit_attn(512, 512, 0, fillers=proj_group(2, kv_ns=[2, 3]))
            emit_attn(512, 512, 1, fillers=proj_group(3))
            emit_attn(1024, 512, 0, fillers=outproj_group(0))
            emit_attn(1024, 512, 1, fillers=outproj_group(1))
            og2 = outproj_group(2)
            emit_attn(1536, 512, 0, fillers=og2[:8])
            # final window: first-half outproj rides as late fillers
            # (valid once the first-half normalize lands); only the
            # second-half outproj chain sits past the last PV.
            og3a = []
            for i, (m, n) in enumerate((m, n) for m in range(2) for n in range(2)):
                og3a += emit_outproj(1536 + 128 * m, n, st_eng=i, tail=True)
            emit_attn(1536, 512, 1, fillers=og2[8:], late_fillers=og3a)
            for i, (m, n) in enumerate((m, n) for m in range(2) for n in range(2)):
                for u in emit_outproj(1792 + 128 * m, n, st_eng=i, tail=True,
                                      drain_sc=(i % 2 == 1)):
                    u()

            if DEBUG:
                nc.sync.dma_start(out=dbg_kv.ap(), in_=kvT_sb)
                nc.sync.dma_start(
                    out=dbg_k.ap().rearrange("p (c t) -> p c t", c=2), in_=kT_sb)
                nc.sync.dma_start(
                    out=dbg_q.ap().rearrange("p (c t) -> p c t", c=2), in_=qT_sb)
                nc.sync.dma_start(
                    out=dbg_yn.ap().rearrange("p (c t) -> p c t", c=2), in_=ynT_sb)
                nc.sync.dma_start(
                    out=dbg_v.ap().rearrange("p (t h d) -> p t h d",
                                             t=NKT, h=HPC), in_=v_sb)

    nc.compile()
    return nc


def _prep_inputs(inputs):
    """Host-side shard prep: per-core input dicts."""
    x = np.asarray(inputs["x"], np.float32)
    Wc = np.asarray(inputs["Wc"], np.float32)
    bc = np.asarray(inputs["bc"], np.float32)
    Wk = np.asarray(inputs["Wk"], np.float32)
    Wv = np.asarray(inputs["Wv"], np.float32)
    bv = np.asarray(inputs["bv"], np.float32)
    Wq = np.asarray(inputs["Wq"], np.float32)
    bq = np.asarray(inputs["bq"], np.float32)
    Wo = np.asarray(inputs["Wo"], np.float32)

    tri = np.triu(np.ones((128, 128), np.float32)).astype(BF16)  # key r <= q c
    wc_b = np.ascontiguousarray(
        Wc.reshape(8, 128, KV).transpose(1, 0, 2).reshape(128, 8 * KV)).astype(BF16)
    bc_b = bc.reshape(KV, 1).astype(np.float32)

    xT = [np.ascontiguousarray(x[b].T).astype(BF16) for b in range(B)]

    in_maps = []
    for core in range(8):
        b, g = core // 4, core % 4
        gsl = slice(g * GD, (g + 1) * GD)
        in_maps.append({
            "xT": xT[b],
            "wq": np.ascontiguousarray(Wq[:, gsl] * SCALE).astype(BF16),
            "bq": np.ascontiguousarray((bq[gsl] * SCALE).reshape(2, 128).T).astype(np.float32),
            "wc": wc_b,
            "bc": bc_b,
            "wk": np.ascontiguousarray(Wk[:, gsl]).astype(BF16),
            "wv": np.ascontiguousarray(Wv[:, gsl]).astype(BF16),
            "bv": np.ascontiguousarray(bv[gsl]).reshape(1, GD).astype(BF16),
            "wo": np.ascontiguousarray(Wo[gsl, :]).astype(BF16),
            "tri": tri,
        })
    return in_maps


def run(inputs, trace=False, tmpdir=None):
    if "nc" not in _CACHE:
        _CACHE["nc"] = _build_program()
    nc = _CACHE["nc"]
    in_maps = _prep_inputs(inputs)

    kwargs = {}
    if trace:
        # NTFF profiling under axon needs the antenv.axon_hooks bridge;
        # shim it if the image lacks it.
        try:
            import antenv.axon_hooks  # noqa: F401
        except ImportError:
            import types
            import antenv  # noqa: F401
            from trn_agent_boot.trn_boot import _ntff_profile_via_ctypes
            hook = _ntff_profile_via_ctypes("/opt/axon/libaxon_pjrt.so")
            mod = types.ModuleType("antenv.axon_hooks")
            mod.get_axon_ntff_profile_hook = lambda: hook
            sys.modules["antenv.axon_hooks"] = mod
        kwargs = dict(trace=True, tmpdir=tmpdir)

    res = run_bass_kernel_spmd(nc, in_maps, list(range(8)), **kwargs)

    bo = np.asarray(inputs["bo"], np.float32)
    out = np.zeros((B, T, D), np.float32)
    for core in range(8):
        out[core // 4] += res.results[core]["outp"].astype(np.float32)
    out += bo
    return out, res


def kernel(**inputs):
    out, _ = run(inputs, trace=False)
    return out


# revision 32
# speedup vs baseline: 1.0059x; 1.0059x over previous
"""Multi-Head Latent Attention kernel for 8 Trainium2 NeuronCores.

Sharding: 8 cores = 2 (batch) x 4 (head groups of 4 heads).
Each core computes, for its (batch b, head group g):
  - kv = x_b @ Wc + bc              (replicated small compressor)
  - k,v,q projections for its 4 heads (column-parallel)
  - causal attention for its 4 heads (transpose-free: S^T layout)
  - partial out = y_heads @ Wo[rows of g]   (row-parallel)
Host sums the 4 partials per batch and adds bo.

All matmuls run in bf16 with fp32 PSUM accumulation. Softmax runs
without max-subtraction (scores for this problem are O(1)). The
denominator is REPLICATED across 64 PSUM partitions for free by
augmenting V with a 64-wide block of ones columns (M=128 instead of
65; matmul time depends only on streamed columns), so the normalize
is reciprocal + muls with no cross-partition broadcast. k-bias is
dropped entirely (softmax is invariant to per-query constants:
q.bk is constant across keys).

Scheduling: all projection/out-proj work is split into ~single-matmul
micro-closures drip-fed between the scores and PV matmuls of the
attention ki-loop, keeping PE dense (p-state ramps to full clock).
The normalize is column-split: the first half of each query window is
final a few key-tiles before the window ends, so only a short
second-half chain sits on the window boundary.
"""
import sys
import math

sys.path.insert(0, "/opt/trn_rl_repo")

import numpy as np
import ml_dtypes

import concourse.bass as bass
import concourse.tile as tile
from concourse import bacc, mybir
from concourse.bass_utils import run_bass_kernel_spmd

BF16 = ml_dtypes.bfloat16

# Problem shape (hardcoded per contract)
B, T, D = 2, 2048, 1024
H = 16
HD = 64           # head dim
KV = 16           # latent dim
HPC = 4           # heads per core
GD = HPC * HD     # head-group width = 256
NKT = T // 128    # key tiles = 16
SCALE = 1.0 / math.sqrt(HD)

F32 = mybir.dt.float32
BF = mybir.dt.bfloat16

_CACHE = {}
DEBUG = False


def _build_program():
    nc = bacc.Bacc("TRN2", target_bir_lowering=False, debug=False)

    xT = nc.dram_tensor("xT", [D, T], BF, kind="ExternalInput")
    wq = nc.dram_tensor("wq", [D, GD], BF, kind="ExternalInput")
    bq = nc.dram_tensor("bq", [128, 2], F32, kind="ExternalInput")
    wc = nc.dram_tensor("wc", [128, 8 * KV], BF, kind="ExternalInput")
    bc = nc.dram_tensor("bc", [KV, 1], F32, kind="ExternalInput")
    wk = nc.dram_tensor("wk", [KV, GD], BF, kind="ExternalInput")
    wv = nc.dram_tensor("wv", [KV, GD], BF, kind="ExternalInput")
    bv = nc.dram_tensor("bv", [1, GD], BF, kind="ExternalInput")
    wo = nc.dram_tensor("wo", [GD, D], BF, kind="ExternalInput")
    tri = nc.dram_tensor("tri", [128, 128], BF, kind="ExternalInput")
    outp = nc.dram_tensor("outp", [T, D], BF, kind="ExternalOutput")
    if DEBUG:
        dbg_kv = nc.dram_tensor("dbg_kv", [KV, T], BF, kind="ExternalOutput")
        dbg_k = nc.dram_tensor("dbg_k", [128, 2 * T], BF, kind="ExternalOutput")
        dbg_q = nc.dram_tensor("dbg_q", [128, 2 * T], BF, kind="ExternalOutput")
        dbg_yn = nc.dram_tensor("dbg_yn", [128, 2 * T], BF, kind="ExternalOutput")
        dbg_v = nc.dram_tensor("dbg_v", [128, NKT * HPC * 128], BF,
                               kind="ExternalOutput")

    EXP = mybir.ActivationFunctionType.Exp

    with tile.TileContext(nc) as tc:
        with (
            tc.tile_pool(name="const", bufs=1) as const,
            tc.tile_pool(name="work", bufs=3) as work,
            tc.tile_pool(name="pexps", bufs=16) as pexps,
            tc.tile_pool(name="ostg", bufs=4) as ostg,
            tc.tile_pool(name="ps", bufs=2, space="PSUM") as ps,
        ):
            # ---- DMA issue plan. kv(0) needs only wc + xts[0]; its drain
            # needs bc; k needs wk; q needs wq+bq; v needs wv+bv; diag
            # needs tri. Priority-ordered so the kv chain starts ~3us in.
            # scalar only issues tri+bv (so exp can start ~1.5us in);
            # gpsimd only slab0-odd + wq (so tri-muls start early).
            wc_sb = const.tile([128, 8, KV], BF)
            nc.sync.dma_start(out=wc_sb, in_=wc.ap().rearrange("p (k m) -> p k m", m=KV))

            xts = []
            xT_r = xT.ap().rearrange("(k p) t -> p k t", p=128)
            for n in range(4):
                xts.append(const.tile([128, 8, 512], BF, name=f"xts{n}"))
            for kt in range(8):
                eng = nc.sync if kt % 2 == 0 else nc.gpsimd
                eng.dma_start(out=xts[0][:, kt, :], in_=xT_r[:, kt, 0:512])
            wq_sb = const.tile([128, 8, GD], BF)
            nc.gpsimd.dma_start(
                out=wq_sb, in_=wq.ap().rearrange("(k p) m -> p k m", p=128))

            bc_sb = const.tile([KV, 1], F32)
            nc.sync.dma_start(out=bc_sb, in_=bc.ap())
            wk_sb = const.tile([KV, GD], BF)
            nc.sync.dma_start(out=wk_sb, in_=wk.ap())
            wv_sb = const.tile([KV, GD], BF)
            nc.sync.dma_start(out=wv_sb, in_=wv.ap())
            bq_sb = const.tile([128, 2, 1], F32)
            nc.sync.dma_start(out=bq_sb, in_=bq.ap().rearrange("p (c o) -> p c o", o=1))

            tri_sb = const.tile([128, 128], BF)
            nc.scalar.dma_start(out=tri_sb, in_=tri.ap())
            bvbc_sb = const.tile([128, GD], BF)
            bv_row = bv.ap()
            bv_bcast = bass.AP(tensor=bv_row.tensor, offset=bv_row.offset,
                               ap=[[0, 128]] + list(bv_row.ap)[1:])
            nc.scalar.dma_start(out=bvbc_sb, in_=bv_bcast)

            for kt in range(8):
                eng = nc.sync if kt % 2 == 0 else nc.gpsimd
                eng.dma_start(out=xts[1][:, kt, :], in_=xT_r[:, kt, 512:1024])
            for kt in range(0, 8, 2):
                nc.sync.dma_start(
                    out=xts[2][:, kt:kt + 2, :],
                    in_=xT_r[:, kt:kt + 2, 1024:1536])
            for kt in range(0, 8, 2):
                nc.sync.dma_start(
                    out=xts[3][:, kt:kt + 2, :],
                    in_=xT_r[:, kt:kt + 2, 1536:2048])
            wo_sb = const.tile([128, 2, D], BF)
            wo_r = wo.ap().rearrange("(k p) n -> p k n", p=128)
            for kt in range(2):
                nc.sync.dma_start(out=wo_sb[:, kt, :], in_=wo_r[:, kt, :])

            kvT_sb = const.tile([KV, T], BF)
            kT_sb = const.tile([128, 2, T], BF)
            qT_sb = const.tile([128, 2, T], BF)
            ynT_sb = const.tile([128, 2, T], BF)
            # v_sb per (key-tile, head): [v(64) | ones(64)] so the PV
            # matmul (M=128) lands y on partitions 0:64 and the softmax
            # denominator REPLICATED on partitions 64:128.
            v_sb = const.tile([128, NKT, HPC, 128], BF)
            nc.vector.memset(v_sb[:, 0:4, :, HD:128], 1.0)
            nc.gpsimd.memset(v_sb[:, 4:16, :, HD:128], 1.0)

            # ---- micro-unit emitters: each returns a list of closures,
            # ~one 512-col matmul each. PSUM tiles allocate at fire time.
            def emit_kv(ns_):
                """kv compress for one or two 512-token slabs. With two,
                the slabs pack at PE col positions 0/32 (M=16 each) and
                their streams run concurrently (col tiling)."""
                cell = {}

                def mm(kt):
                    if kt == 0:
                        cell["p"] = ps.tile([128, 512], F32, tag="bank",
                                            bufs=2, name=f"pkv{ns_[0]}")
                    for j, n in enumerate(ns_):
                        nc.tensor.matmul(
                            cell["p"][32 * j:32 * j + KV, :],
                            lhsT=wc_sb[:, kt, :], rhs=xts[n][:, kt, :],
                            start=(kt == 0), stop=(kt == 7),
                            tile_position=(0, 32 * j),
                        )
                    if kt == 7:
                        for j, n in enumerate(ns_):
                            nc.vector.tensor_scalar_add(
                                kvT_sb[:, n * 512:(n + 1) * 512],
                                cell["p"][32 * j:32 * j + KV, :], bc_sb)
                return [lambda kt=kt: (mm(kt), mm(kt + 1))
                        for kt in range(0, 8, 2)]

            def emit_k(c, n):
                def mm():
                    ns = slice(n * 512, n * 512 + 512)
                    pk = ps.tile([128, 512], F32, tag="bank", bufs=2,
                                 name=f"pk{c}{n}")
                    nc.tensor.matmul(
                        pk, lhsT=wk_sb[:, c * 128:(c + 1) * 128],
                        rhs=kvT_sb[:, ns], start=True, stop=True,
                    )
                    nc.vector.tensor_copy(out=kT_sb[:, c, ns], in_=pk)
                return [mm]

            def emit_q(c, n):
                cell = {}
                ns = slice(n * 512, n * 512 + 512)

                def mm(kt):
                    if kt == 0:
                        cell["p"] = ps.tile([128, 512], F32, tag="bank",
                                            bufs=2, name=f"pq{c}{n}")
                    nc.tensor.matmul(
                        cell["p"], lhsT=wq_sb[:, kt, c * 128:(c + 1) * 128],
                        rhs=xts[n][:, kt, :], start=(kt == 0), stop=(kt == 7),
                    )
                    if kt == 7:
                        nc.vector.tensor_scalar_add(
                            qT_sb[:, c, ns], cell["p"], bq_sb[:, c, :])
                return [lambda kt=kt: (mm(kt), mm(kt + 1))
                        for kt in range(0, 8, 2)]

            def emit_v(t):
                def mm():
                    pv = ps.tile([128, 512], F32, tag="bank", bufs=2,
                                 name=f"pv{t}")
                    nc.tensor.matmul(
                        pv[:, 0:GD], lhsT=kvT_sb[:, t * 128:(t + 1) * 128],
                        rhs=wv_sb, start=True, stop=True,
                    )
                    nc.vector.tensor_add(
                        out=v_sb[:, t, :, 0:HD],
                        in0=pv[:, 0:GD].rearrange("p (h d) -> p h d", h=HPC),
                        in1=bvbc_sb.rearrange("p (h d) -> p h d", h=HPC),
                    )
                return [mm]

            MID_ENGS = [nc.sync, nc.gpsimd]
            TAIL_ENGS = [nc.sync, nc.gpsimd, nc.scalar]

            def emit_outproj(qs, n, st_eng=0, tail=False, drain_sc=False):
                """out rows [qs, qs+128), cols [n*512, (n+1)*512): two
                micro-units (one matmul each; drain+store on the second)."""
                cell = {}

                def mm0():
                    cell["p"] = ps.tile([128, 512], F32, tag="bank", bufs=2,
                                        name=f"po{qs}_{n}")
                    nc.tensor.matmul(
                        cell["p"], lhsT=ynT_sb[:, 0, qs:qs + 128],
                        rhs=wo_sb[:, 0, n * 512:(n + 1) * 512],
                        start=True, stop=False,
                    )

                def mm1():
                    po = cell["p"]
                    nc.tensor.matmul(
                        po, lhsT=ynT_sb[:, 1, qs:qs + 128],
                        rhs=wo_sb[:, 1, n * 512:(n + 1) * 512],
                        start=False, stop=True,
                    )
                    st = ostg.tile([128, 512], BF, tag="ostg",
                                   name=f"ost{qs}_{n}")
                    if drain_sc:
                        nc.scalar.copy(st, po)
                    else:
                        nc.vector.tensor_copy(st, po)
                    engs = TAIL_ENGS if tail else MID_ENGS
                    engs[st_eng % len(engs)].dma_start(
                        out=outp.ap()[qs:qs + 128, n * 512:(n + 1) * 512],
                        in_=st,
                    )
                return [mm0, mm1]

            tri_ctr = [0]

            def emit_attn(q0, qw, pair, fillers=None, late_groups=None,
                          tail_fillers=None, n_splits=2):
                """Causal attention for queries [q0, q0+qw), heads 2*pair,
                2*pair+1. fillers: micro-closures paced across the ki loop
                between scores and PV. The normalize is split into
                n_splits column pieces, each emitted as soon as its query
                columns stop receiving PV contributions; late_groups[j]
                joins the paced queue right after piece j's normalize."""
                fillers = list(fillers or [])
                late_groups = dict(late_groups or {})
                n_ki = (q0 + qw) // 128
                seg = qw // n_splits
                # piece j covers cols [j*seg,(j+1)*seg); final after this ki
                ki_done = {n_ki - 1 - (qw - (j + 1) * seg) // 128: j
                           for j in range(n_splits - 1)}
                tag = f"{q0}_{pair}"
                y_ps = ps.tile([128, 2, 512], F32, tag="yb", bufs=1,
                               name=f"yps{tag}")

                def normalize(c0, c1):
                    dn = work.tile([64, 2, 512], F32, tag="dn",
                                   name=f"dn{tag}_{c0}")
                    nc.vector.tensor_copy(
                        out=dn[:, :, c0:c1], in_=y_ps[64:128, :, c0:c1])
                    rbc = work.tile([64, 2, 512], F32, tag="rbc",
                                    name=f"rbc{tag}_{c0}")
                    nc.vector.reciprocal_approx_fast(
                        out=rbc[:, :, c0:c1], in_=dn[:, :, c0:c1])
                    for h in range(2):
                        nc.vector.tensor_mul(
                            ynT_sb[h * 64:(h + 1) * 64, pair,
                                   q0 + c0:q0 + c1],
                            y_ps[0:64, h, c0:c1], rbc[:, h, c0:c1],
                        )

                total = len(fillers)
                popped = 0
                pace_den = max(1, n_ki - 1)
                prev = None
                for ki in range(n_ki):
                    vs = max(0, 128 * ki - q0)
                    s_ps = ps.tile([128, 2, 512], F32, tag="sc", bufs=2,
                                   name=f"s{tag}_{ki}")
                    for h in range(2):
                        base = h * 64
                        nc.tensor.matmul(
                            s_ps[:, h, vs:qw],
                            lhsT=kT_sb[base:base + 64, pair,
                                       ki * 128:(ki + 1) * 128],
                            rhs=qT_sb[base:base + 64, pair, q0 + vs:q0 + qw],
                            start=True, stop=True,
                            tile_position=(base, 0),
                        )
                    px = pexps.tile([128, 2, 512], BF, tag="pexp",
                                    name=f"px{tag}_{ki}")
                    nc.scalar.activation(px[:, :, vs:qw], s_ps[:, :, vs:qw], EXP)
                    if ki * 128 >= q0:
                        tri_b = bass.AP(tensor=tri_sb.tensor, offset=tri_sb.offset,
                                        ap=[list(tri_sb.ap)[0], [0, 2], [1, 128]])
                        eng = nc.gpsimd if tri_ctr[0] % 2 == 0 else nc.vector
                        tri_ctr[0] += 1
                        eng.tensor_mul(
                            px[:, :, vs:vs + 128],
                            px[:, :, vs:vs + 128], tri_b,
                        )
                    # paced micro-fillers between scores and the lagged PV
                    # (capped per slot so a burst never delays the PV)
                    tgt = min((total * (ki + 1) + pace_den - 1) // pace_den,
                              popped + 3, total)
                    while popped < tgt:
                        fillers.pop(0)()
                        popped += 1
                    if prev is not None:
                        pki, ppx, pvs = prev
                        for h in range(2):
                            nc.tensor.matmul(
                                y_ps[:, h, pvs:qw],
                                lhsT=v_sb[:, pki, 2 * pair + h, :],
                                rhs=ppx[:, h, pvs:qw],
                                start=(pki == 0), stop=False,
                            )
                        if pki in ki_done:
                            j = ki_done[pki]
                            normalize(j * seg, (j + 1) * seg)
                            # late fillers join the paced queue (they
                            # depend on this normalize piece)
                            lg = late_groups.pop(j, [])
                            fillers.extend(lg)
                            total += len(lg)
                    prev = (ki, px, vs)
                pki, ppx, pvs = prev
                for h in range(2):
                    nc.tensor.matmul(
                        y_ps[:, h, pvs:qw],
                        lhsT=v_sb[:, pki, 2 * pair + h, :],
                        rhs=ppx[:, h, pvs:qw],
                        start=(pki == 0), stop=True,
                    )
                normalize(qw - seg, qw)
                for f in fillers:       # leftovers past the pop cap
                    f()
                for f in (tail_fillers or []):
                    f()

            # ---- emission schedule ----
            def proj_group(n, kv_ns=None):
                units = emit_kv(kv_ns) if kv_ns else []
                units += emit_k(0, n) + emit_k(1, n)
                units += emit_q(0, n) + emit_q(1, n)
                for t in range(4 * n, 4 * n + 4):
                    units += emit_v(t)
                return units

            def outproj_group(qc):
                units = []
                for i, (m, n) in enumerate((m, n) for m in range(4)
                                           for n in range(2)):
                    units += emit_outproj(qc * 512 + m * 128, n, st_eng=i)
                return units

            for u in proj_group(0, kv_ns=[0]):
                u()
            g1 = proj_group(1, kv_ns=[1])
            emit_attn(0, 512, 0, fillers=g1[:9])
            emit_attn(0, 512, 1, fillers=g1[9:])
            emit_attn(512, 512, 0, fillers=proj_group(2, kv_ns=[2, 3]))
            emit_attn(512, 512, 1, fillers=proj_group(3))
            emit_attn(1024, 512, 0, fillers=outproj_group(0))
            emit_attn(1024, 512, 1, fillers=outproj_group(1))
            og2 = outproj_group(2)
            emit_attn(1536, 512, 0, fillers=og2[:8])
            # final window: quarter-split normalize; each out-proj row
            # block rides as a late filler of its quarter, so only the
            # last 128-query chain sits past the final PV.
            late = {}
            for m in range(3):
                units = []
                for i, n in enumerate(range(2)):
                    units += emit_outproj(1536 + 128 * m, n, st_eng=m * 2 + i,
                                          tail=True)
                late[m] = units
            emit_attn(1536, 512, 1, fillers=og2[8:], late_groups=late,
                      n_splits=4)
            for i, n in enumerate(range(2)):
                for u in emit_outproj(1920, n, st_eng=i, tail=True,
                                      drain_sc=(i % 2 == 1)):
                    u()

            if DEBUG:
                nc.sync.dma_start(out=dbg_kv.ap(), in_=kvT_sb)
                nc.sync.dma_start(
                    out=dbg_k.ap().rearrange("p (c t) -> p c t", c=2), in_=kT_sb)
                nc.sync.dma_start(
                    out=dbg_q.ap().rearrange("p (c t) -> p c t", c=2), in_=qT_sb)
                nc.sync.dma_start(
                    out=dbg_yn.ap().rearrange("p (c t) -> p c t", c=2), in_=ynT_sb)
                nc.sync.dma_start(
                    out=dbg_v.ap().rearrange("p (t h d) -> p t h d",
                                             t=NKT, h=HPC), in_=v_sb)

    nc.compile()
    return nc


def _prep_inputs(inputs):
    """Host-side shard prep: per-core input dicts."""
    x = np.asarray(inputs["x"], np.float32)
    Wc = np.asarray(inputs["Wc"], np.float32)
    bc = np.asarray(inputs["bc"], np.float32)
    Wk = np.asarray(inputs["Wk"], np.float32)
    Wv = np.asarray(inputs["Wv"], np.float32)
    bv = np.asarray(inputs["bv"], np.float32)
    Wq = np.asarray(inputs["Wq"], np.float32)
    bq = np.asarray(inputs["bq"], np.float32)
    Wo = np.asarray(inputs["Wo"], np.float32)

    tri = np.triu(np.ones((128, 128), np.float32)).astype(BF16)  # key r <= q c
    wc_b = np.ascontiguousarray(
        Wc.reshape(8, 128, KV).transpose(1, 0, 2).reshape(128, 8 * KV)).astype(BF16)
    bc_b = bc.reshape(KV, 1).astype(np.float32)

    xT = [np.ascontiguousarray(x[b].T).astype(BF16) for b in range(B)]

    in_maps = []
    for core in range(8):
        b, g = core // 4, core % 4
        gsl = slice(g * GD, (g + 1) * GD)
        in_maps.append({
            "xT": xT[b],
            "wq": np.ascontiguousarray(Wq[:, gsl] * SCALE).astype(BF16),
            "bq": np.ascontiguousarray((bq[gsl] * SCALE).reshape(2, 128).T).astype(np.float32),
            "wc": wc_b,
            "bc": bc_b,
            "wk": np.ascontiguousarray(Wk[:, gsl]).astype(BF16),
            "wv": np.ascontiguousarray(Wv[:, gsl]).astype(BF16),
            "bv": np.ascontiguousarray(bv[gsl]).reshape(1, GD).astype(BF16),
            "wo": np.ascontiguousarray(Wo[gsl, :]).astype(BF16),
            "tri": tri,
        })
    return in_maps


def run(inputs, trace=False, tmpdir=None):
    if "nc" not in _CACHE:
        _CACHE["nc"] = _build_program()
    nc = _CACHE["nc"]
    in_maps = _prep_inputs(inputs)

    kwargs = {}
    if trace:
        # NTFF profiling under axon needs the antenv.axon_hooks bridge;
        # shim it if the image lacks it.
        try:
            import antenv.axon_hooks  # noqa: F401
        except ImportError:
            import types
            import antenv  # noqa: F401
            from trn_agent_boot.trn_boot import _ntff_profile_via_ctypes
            hook = _ntff_profile_via_ctypes("/opt/axon/libaxon_pjrt.so")
            mod = types.ModuleType("antenv.axon_hooks")
            mod.get_axon_ntff_profile_hook = lambda: hook
            sys.modules["antenv.axon_hooks"] = mod
        kwargs = dict(trace=True, tmpdir=tmpdir)

    res = run_bass_kernel_spmd(nc, in_maps, list(range(8)), **kwargs)

    bo = np.asarray(inputs["bo"], np.float32)
    out = np.zeros((B, T, D), np.float32)
    for core in range(8):
        out[core // 4] += res.results[core]["outp"].astype(np.float32)
    out += bo
    return out, res


def kernel(**inputs):
    out, _ = run(inputs, trace=False)
    return out
